# revision 8
# baseline (speedup 1.0000x reference)
"""Expert-parallel MoE (top-2 of 8 experts) for 8 Trainium2 NeuronCores.

Problem (hardcoded): x [4,2048,1024] f32, D=1024, H=4096, E=8, K=2.
Sharding: core c owns token shard c (1024 tokens: routing, dispatch,
final combine) AND expert c (FFN weights W1[c]/b1[c]/W2[c]/b2[c]).

Per-core pipeline:
  1. Load + PE-transpose own x shard; fp32 router matmul -> logits;
     softmax -> gating output; max8 -> top-2 idx/vals; renormalized
     top-2 probs; per-expert prefix-sum (triangular matmuls) gives each
     (token, k) a slot in the dispatch buffer: slot = expert*CP + pos.
  2. Indirect-scatter x rows into send1[8*CP, 1024]; AllToAll.
  3. FFN on recv1 rows (fp32r matmuls): hT = relu(W1^T xg^T + b1),
     y = hT^T W2 + b2 -> send2; AllToAll back.
  4. Gather own tokens' two y rows at known slots; out = p1*y1 + p2*y2.

Outputs per core: out shard, gating shard, top-idx shard; host concats.
"""

import numpy as np

import concourse.bacc as bacc
import concourse.bass as bass
import concourse.mybir as mybir
from concourse.bass import IndirectOffsetOnAxis
from concourse.tile import TileContext

P = 128
D = 1024
H = 4096
E = 8
NSH = 1024          # tokens per core (shard)
NT = NSH // P       # 8 token tiles per shard
DT = D // P         # 8 contraction tiles over D
HT = H // P         # 32 tiles over H
CP = 320            # dispatch capacity per (owner, expert) pair
ROWS = E * CP       # 2560 rows in dispatch buffers
BLK = 640           # ffn block: tokens per block
NBT = BLK // P      # 5 token tiles per block
NBLK = ROWS // BLK  # 4 blocks
NG = 320            # moving-N per matmul1 group (2 groups/block, >=256 for f32r)
DH = 512            # D-half for matmul2 moving N

F32 = mybir.dt.float32
F32R = mybir.dt.float32r
I32 = mybir.dt.int32
U32 = mybir.dt.uint32
AX = mybir.AxisListType
ALU = mybir.AluOpType
ACT = mybir.ActivationFunctionType


def r(ap):
    return ap.bitcast(F32R)


def build_nc():
    nc = bacc.Bacc(None, target_bir_lowering=False)

    x_sh = nc.dram_tensor("x_shard", [NSH, D], F32, kind="ExternalInput")
    rw = nc.dram_tensor("router_W", [D, E], F32, kind="ExternalInput")
    rb = nc.dram_tensor("router_b", [1, E], F32, kind="ExternalInput")
    w1 = nc.dram_tensor("W1", [D, H], F32, kind="ExternalInput")
    b1 = nc.dram_tensor("b1", [H], F32, kind="ExternalInput")
    w2 = nc.dram_tensor("W2", [H, D], F32, kind="ExternalInput")
    b2 = nc.dram_tensor("b2", [1, D], F32, kind="ExternalInput")

    out_sh = nc.dram_tensor("out_shard", [NSH, D], F32, kind="ExternalOutput")
    gat_sh = nc.dram_tensor("gating_shard", [NSH, E], F32, kind="ExternalOutput")
    tid_sh = nc.dram_tensor("topidx_shard", [NSH, 2], I32, kind="ExternalOutput")

    send1 = nc.dram_tensor("send1", [ROWS, D], F32)
    recv1 = nc.dram_tensor("recv1", [ROWS, D], F32)
    send2 = nc.dram_tensor("send2", [ROWS, D], F32)
    recv2 = nc.dram_tensor("recv2", [ROWS, D], F32)

    ident_c = nc.inline_tensor(np.eye(P, dtype=np.float32), name="ident")
    # lt[j, i] = 1 iff j < i : (lt.T @ m)[i] = sum_{j<i} m[j]  (exclusive)
    lt_c = nc.inline_tensor(np.triu(np.ones((P, P), np.float32), 1), name="lt")
    blk = np.zeros((E * NT, E * NT), np.float32)
    for e in range(E):
        s = slice(e * NT, (e + 1) * NT)
        blk[s, s] = np.triu(np.ones((NT, NT), np.float32), 1)
    blk_c = nc.inline_tensor(blk, name="blktri")
    ones_row_c = nc.inline_tensor(np.ones((1, P), np.float32), name="ones_row")
    ones_col_c = nc.inline_tensor(np.ones((P, 1), np.float32), name="ones_col")

    rg = [list(range(8))]

    with TileContext(nc) as tc:
        with (
            tc.tile_pool(name="const", bufs=1) as cpool,
            tc.tile_pool(name="small", bufs=1) as spool,
            tc.tile_pool(name="tps", bufs=1, space="PSUM") as tpsum,
        ):
            # ---- constants into SBUF ----
            id_sb = cpool.tile_from(ident_c[:, :])
            lt_sb = cpool.tile_from(lt_c[:, :])
            blk_sb = cpool.tile_from(blk_c[:, :])
            ones_row = cpool.tile_from(ones_row_c[:, :])
            ones_col = cpool.tile_from(ones_col_c[:, :])
            rb_sb = cpool.tile_from(rb[:, :])
            b2_sb = cpool.tile_from(b2[:, :])
            rw_sb = cpool.tile([P, DT * E], F32)
            nc.sync.dma_start(rw_sb[:, :], rw.rearrange("(dt p) e -> p dt e", p=P))
            b1_sb = cpool.tile([P, HT], F32)
            nc.sync.dma_start(b1_sb[:, :], b1.rearrange("(ht p) -> p ht", p=P))

            # ---- persistent routing state (alive through phase 5) ----
            p1_sb = spool.tile([P, NT], F32, tag="p1")
            p2_sb = spool.tile([P, NT], F32, tag="p2")
            idx1f = spool.tile([P, NT], F32, tag="idx1f")
            idx2f = spool.tile([P, NT], F32, tag="idx2f")
            mask_sb = spool.tile([P, E * NT], F32, tag="mask")
            slot1_i = spool.tile([P, NT], I32, tag="slot1i")
            slot2_i = spool.tile([P, NT], I32, tag="slot2i")

            # =======================================================
            # Phase 1: load + transpose x shard, router, routing state
            # =======================================================
            with (
                tc.tile_pool(name="ph1", bufs=1) as xpool,
                tc.tile_pool(name="ph1s", bufs=2) as wpool,
                tc.tile_pool(name="mmps", bufs=1, space="PSUM") as mpsum,
            ):
                # Note: send1/send2 pad slots (beyond each pair's token count)
                # are never written and never consumed: garbage there stays
                # column/row-local through both matmuls and is never gathered.
                x_tiles = []
                xT = [xpool.tile([P, NSH], F32, tag=f"xT{d}", name=f"xT{d}") for d in range(DT)]
                for t in range(NT):
                    xt = xpool.tile([P, D], F32, tag=f"x{t}")
                    x_tiles.append(xt)
                    nc.sync.dma_start(xt[:, :], x_sh[t * P:(t + 1) * P, :])
                    for d in range(DT):
                        ps = tpsum.tile([P, P], F32)
                        nc.tensor.transpose(
                            ps[:, :], xt[:, d * P:(d + 1) * P], id_sb[:, :]
                        )
                        nc.vector.tensor_copy(xT[d][:, t * P:(t + 1) * P], ps[:, :])

                for t in range(NT):
                    tsl = slice(t * P, (t + 1) * P)
                    lps = mpsum.tile([P, E], F32, tag="lps")
                    for d in range(DT):
                        nc.tensor.matmul(
                            lps[:, :],
                            lhsT=xT[d][:, tsl],
                            rhs=rw_sb[:, d * E:(d + 1) * E],
                            start=(d == 0),
                            stop=False,
                        )
                    nc.tensor.matmul(
                        lps[:, :], lhsT=ones_row[:1, :], rhs=rb_sb[:1, :],
                        start=False, stop=True,
                    )
                    logit = wpool.tile([P, E], F32, tag="logit")
                    nc.vector.tensor_copy(logit[:, :], lps[:, :])

                    vals = wpool.tile([P, 8], F32, tag="vals")
                    idxs = wpool.tile([P, 8], U32, tag="idxs")
                    nc.vector.max_with_indices(vals[:, :], idxs[:, :], logit[:, :])

                    # gating_probs = softmax(logits) ; max is vals[:, 0]
                    nm1 = wpool.tile([P, 1], F32, tag="nm1")
                    nc.vector.tensor_scalar_mul(nm1[:, :], vals[:, 0:1], -1.0)
                    exps = wpool.tile([P, E], F32, tag="exps")
                    nc.scalar.activation(
                        exps[:, :], logit[:, :], ACT.Exp, bias=nm1[:, :1], scale=1.0
                    )
                    ssum = wpool.tile([P, 1], F32, tag="ssum")
                    nc.vector.reduce_sum(ssum[:, :], exps[:, :], axis=AX.X)
                    rsum = wpool.tile([P, 1], F32, tag="rsum")
                    nc.vector.reciprocal(rsum[:, :], ssum[:, :])
                    gat = wpool.tile([P, E], F32, tag="gat")
                    nc.vector.tensor_scalar_mul(gat[:, :], exps[:, :], rsum[:, :1])
                    nc.sync.dma_start(gat_sh[tsl, :], gat[:, :])

                    tid = wpool.tile([P, 2], I32, tag="tid")
                    nc.vector.tensor_copy(tid[:, :], idxs[:, 0:2])
                    nc.sync.dma_start(tid_sh[tsl, :], tid[:, :])

                    # renormalized top-2 probs: e2 = exp(v2 - v1)
                    dif = wpool.tile([P, 1], F32, tag="dif")
                    nc.vector.tensor_tensor(
                        dif[:, :], vals[:, 1:2], vals[:, 0:1], op=ALU.subtract
                    )
                    e2 = wpool.tile([P, 1], F32, tag="e2")
                    nc.scalar.activation(e2[:, :], dif[:, :], ACT.Exp)
                    den = wpool.tile([P, 1], F32, tag="den")
                    nc.vector.tensor_scalar_add(den[:, :], e2[:, :], 1.0)
                    rden = wpool.tile([P, 1], F32, tag="rden")
                    nc.vector.reciprocal(rden[:, :], den[:, :])
                    nc.vector.tensor_copy(p1_sb[:, t:t + 1], rden[:, :])
                    nc.vector.tensor_tensor(
                        p2_sb[:, t:t + 1], e2[:, :], rden[:, :], op=ALU.mult
                    )
                    nc.vector.tensor_copy(idx1f[:, t:t + 1], idxs[:, 0:1])
                    nc.vector.tensor_copy(idx2f[:, t:t + 1], idxs[:, 1:2])

                # masks: mask[:, e*NT + t] = (idx1==e) + (idx2==e)
                for e in range(E):
                    esl = slice(e * NT, (e + 1) * NT)
                    eq1 = wpool.tile([P, NT], F32, tag="eq1")
                    eq2 = wpool.tile([P, NT], F32, tag="eq2")
                    nc.vector.tensor_scalar(
                        eq1[:, :], idx1f[:, :], float(e), None, op0=ALU.is_equal
                    )
                    nc.vector.tensor_scalar(
                        eq2[:, :], idx2f[:, :], float(e), None, op0=ALU.is_equal
                    )
                    nc.vector.tensor_tensor(
                        mask_sb[:, esl], eq1[:, :], eq2[:, :], op=ALU.add
                    )

                # prefix sums -> pos (exclusive, token order n = t*128 + p)
                cs_ps = mpsum.tile([E * NT, 1], F32, tag="cs")
                nc.tensor.matmul(
                    cs_ps[:, :], lhsT=mask_sb[:, :], rhs=ones_col[:, :1],
                    start=True, stop=True,
                )
                cs_sb = wpool.tile([E * NT, 1], F32, tag="cs_sb")
                nc.vector.tensor_copy(cs_sb[:, :], cs_ps[:, :])
                co_ps = mpsum.tile([E * NT, 1], F32, tag="co")
                nc.tensor.matmul(
                    co_ps[:, :], lhsT=blk_sb[:, :], rhs=cs_sb[:, :1],
                    start=True, stop=True,
                )
                co_sb = wpool.tile([E * NT, 1], F32, tag="co_sb")
                nc.vector.tensor_copy(co_sb[:, :], co_ps[:, :])
                cot_ps = mpsum.tile([1, E * NT], F32, tag="cot")
                nc.tensor.transpose(
                    cot_ps[:1, :], co_sb[:, :1], id_sb[:E * NT, :E * NT]
                )
                cot_sb = wpool.tile([1, E * NT], F32, tag="cot_sb")
                nc.vector.tensor_copy(cot_sb[:1, :], cot_ps[:1, :])

                pos_ps = mpsum.tile([P, E * NT], F32, tag="pos")
                nc.tensor.matmul(
                    pos_ps[:, :], lhsT=lt_sb[:, :], rhs=mask_sb[:, :],
                    start=True, stop=False,
                )
                nc.tensor.matmul(
                    pos_ps[:, :], lhsT=ones_row[:1, :], rhs=cot_sb[:1, :],
                    start=False, stop=True,
                )
                pos_sb = wpool.tile([P, E * NT], F32, tag="pos_sb")
                nc.vector.tensor_copy(pos_sb[:, :], pos_ps[:, :])

                # slots: slot_k = idx_k*CP + pos[:, idx_k block] ; clamp
                for idxf, slot_i in ((idx1f, slot1_i), (idx2f, slot2_i)):
                    slotf = wpool.tile([P, NT], F32, tag="slotf")
                    nc.vector.memset(slotf[:, :], 0.0)
                    for e in range(E):
                        esl = slice(e * NT, (e + 1) * NT)
                        eq = wpool.tile([P, NT], F32, tag="eq")
                        nc.vector.tensor_scalar(
                            eq[:, :], idxf[:, :], float(e), None, op0=ALU.is_equal
                        )
                        pe = wpool.tile([P, NT], F32, tag="pe")
                        nc.vector.tensor_scalar(
                            pe[:, :], pos_sb[:, esl], float(e * CP), None,
                            op0=ALU.add,
                        )
                        nc.vector.tensor_tensor(
                            pe[:, :], eq[:, :], pe[:, :], op=ALU.mult
                        )
                        nc.vector.tensor_tensor(
                            slotf[:, :], slotf[:, :], pe[:, :], op=ALU.add
                        )
                    lim = wpool.tile([P, NT], F32, tag="lim")
                    nc.vector.tensor_scalar(
                        lim[:, :], idxf[:, :], float(CP), float(CP - 1),
                        op0=ALU.mult, op1=ALU.add,
                    )
                    nc.vector.tensor_tensor(
                        slotf[:, :], slotf[:, :], lim[:, :], op=ALU.min
                    )
                    nc.vector.tensor_copy(slot_i[:, :], slotf[:, :])

                # dispatch: scatter x rows to send1[slot]
                for t in range(NT):
                    for slot_i in (slot1_i, slot2_i):
                        nc.gpsimd.indirect_dma_start(
                            out=send1[:, :],
                            out_offset=IndirectOffsetOnAxis(
                                ap=slot_i[:, t:t + 1], axis=0
                            ),
                            in_=x_tiles[t][:, :],
                            in_offset=None,
                        )

            nc.gpsimd.collective_compute(
                "AllToAll", ALU.bypass, ins=[send1[:, :]], outs=[recv1[:, :]],
                replica_groups=rg,
            )

            # =======================================================
            # Phase 3: FFN over recv1 rows (fp32r)
            # =======================================================
            with (
                tc.tile_pool(name="xg", bufs=3) as xgpool,
                tc.tile_pool(name="xgt", bufs=2) as xgtpool,
                tc.tile_pool(name="ht", bufs=1) as htpool,
                tc.tile_pool(name="w1p", bufs=3) as w1pool,
                tc.tile_pool(name="w2p", bufs=3) as w2pool,
                tc.tile_pool(name="yp", bufs=4) as ypool,
                tc.tile_pool(name="ps1", bufs=2, space="PSUM") as ps1pool,
                tc.tile_pool(name="ps2", bufs=1, space="PSUM") as ps2pool,
            ):
                for b in range(NBLK):
                    base = b * BLK
                    xgT = [
                        xgtpool.tile([P, BLK], F32R, tag=f"xgT{d}", name=f"xgT{d}")
                        for d in range(DT)
                    ]
                    for i in range(NBT):
                        xg = xgpool.tile([P, D], F32, tag="xg")
                        nc.sync.dma_start(
                            xg[:, :], recv1[base + i * P:base + (i + 1) * P, :]
                        )
                        for d in range(DT):
                            ps = tpsum.tile([P, P], F32)
                            nc.tensor.transpose(
                                ps[:, :], xg[:, d * P:(d + 1) * P], id_sb[:, :]
                            )
                            nc.vector.tensor_copy(
                                xgT[d][:, i * P:(i + 1) * P], ps[:, :]
                            )

                    hts = [
                        htpool.tile([P, BLK], F32R, tag=f"hT{h}", name=f"hT{h}")
                        for h in range(HT)
                    ]
                    for h in range(HT):
                        w1t = []
                        for d in range(DT):
                            wt = w1pool.tile([P, P], F32R, tag=f"w1_{d % 2}")
                            nc.sync.dma_start(
                                wt[:, :],
                                r(w1[d * P:(d + 1) * P, h * P:(h + 1) * P]),
                            )
                            w1t.append(wt)
                        for g in range(BLK // NG):
                            gsl = slice(g * NG, (g + 1) * NG)
                            ps = ps1pool.tile([P, NG], F32, tag="mm1")
                            for d in range(DT):
                                nc.tensor.matmul(
                                    ps[:, :],
                                    lhsT=w1t[d][:, :],
                                    rhs=xgT[d][:, gsl],
                                    start=(d == 0),
                                    stop=(d == DT - 1),
                                )
                            nc.scalar.activation(
                                hts[h][:, gsl], ps[:, :], ACT.Relu,
                                bias=b1_sb[:, h:h + 1], scale=1.0,
                            )

                    for dh in range(D // DH):
                        dsl = slice(dh * DH, (dh + 1) * DH)
                        yps = [
                            ps2pool.tile([P, DH], F32, tag=f"ys{i}", name=f"ys{i}")
                            for i in range(NBT)
                        ]
                        for h in range(HT):
                            w2t = w2pool.tile([P, DH], F32R, tag="w2")
                            nc.sync.dma_start(
                                w2t[:, :], r(w2[h * P:(h + 1) * P, dsl])
                            )
                            for i in range(NBT):
                                nc.tensor.matmul(
                                    yps[i][:, :],
                                    lhsT=hts[h][:, i * P:(i + 1) * P],
                                    rhs=w2t[:, :],
                                    start=(h == 0),
                                    stop=False,
                                )
                        for i in range(NBT):
                            nc.tensor.matmul(
                                yps[i][:, :], lhsT=ones_row[:1, :],
                                rhs=b2_sb[:1, dsl], start=False, stop=True,
                            )
                            yt = ypool.tile([P, DH], F32, tag="y")
                            nc.vector.tensor_copy(yt[:, :], yps[i][:, :])
                            nc.sync.dma_start(
                                send2[base + i * P:base + (i + 1) * P, dsl],
                                yt[:, :],
                            )

            nc.gpsimd.collective_compute(
                "AllToAll", ALU.bypass, ins=[send2[:, :]], outs=[recv2[:, :]],
                replica_groups=rg,
            )

            # =======================================================
            # Phase 5: combine own tokens
            # =======================================================
            with tc.tile_pool(name="comb", bufs=3) as opool:
                for t in range(NT):
                    tsl = slice(t * P, (t + 1) * P)
                    y1 = opool.tile([P, D], F32, tag="y1")
                    y2 = opool.tile([P, D], F32, tag="y2")
                    nc.gpsimd.indirect_dma_start(
                        out=y1[:, :], out_offset=None, in_=recv2[:, :],
                        in_offset=IndirectOffsetOnAxis(
                            ap=slot1_i[:, t:t + 1], axis=0
                        ),
                    )
                    nc.gpsimd.indirect_dma_start(
                        out=y2[:, :], out_offset=None, in_=recv2[:, :],
                        in_offset=IndirectOffsetOnAxis(
                            ap=slot2_i[:, t:t + 1], axis=0
                        ),
                    )
                    nc.vector.tensor_scalar_mul(y1[:, :], y1[:, :], p1_sb[:, t:t + 1])
                    nc.vector.tensor_scalar_mul(y2[:, :], y2[:, :], p2_sb[:, t:t + 1])
                    nc.vector.tensor_tensor(
                        y1[:, :], y1[:, :], y2[:, :], op=ALU.add
                    )
                    nc.sync.dma_start(out_sh[tsl, :], y1[:, :])

    nc.compile()
    return nc


_NC_CACHE = None


def _get_nc():
    global _NC_CACHE
    if _NC_CACHE is None:
        _NC_CACHE = build_nc()
    return _NC_CACHE


def make_in_maps(x, router_W, router_b, W1, b1, W2, b2):
    x = np.ascontiguousarray(np.asarray(x, dtype=np.float32)).reshape(8 * NSH, D)
    router_W = np.ascontiguousarray(np.asarray(router_W, dtype=np.float32))
    router_b = np.ascontiguousarray(
        np.asarray(router_b, dtype=np.float32).reshape(1, E)
    )
    W1 = np.ascontiguousarray(np.asarray(W1, dtype=np.float32))
    b1 = np.ascontiguousarray(np.asarray(b1, dtype=np.float32))
    W2 = np.ascontiguousarray(np.asarray(W2, dtype=np.float32))
    b2 = np.ascontiguousarray(np.asarray(b2, dtype=np.float32))
    in_maps = []
    for c in range(8):
        in_maps.append({
            "x_shard": x[c * NSH:(c + 1) * NSH],
            "router_W": router_W,
            "router_b": router_b,
            "W1": W1[c],
            "b1": b1[c],
            "W2": W2[c],
            "b2": b2[c].reshape(1, D),
        })
    return in_maps


def assemble(results):
    out = np.concatenate([res["out_shard"] for res in results], axis=0)
    gat = np.concatenate([res["gating_shard"] for res in results], axis=0)
    tid = np.concatenate([res["topidx_shard"] for res in results], axis=0)
    return (
        out.reshape(4, 2048, D).astype(np.float32),
        gat.reshape(4, 2048, E).astype(np.float32),
        tid.reshape(4, 2048, 2).astype(np.int32),
    )


def kernel(x, router_W, router_b, W1, b1, W2, b2):
    from concourse.bass_utils import run_bass_kernel_spmd

    nc = _get_nc()
    in_maps = make_in_maps(x, router_W, router_b, W1, b1, W2, b2)
    res = run_bass_kernel_spmd(nc, in_maps, list(range(8)))
    return assemble(res.results)


# revision 11
# speedup vs baseline: 1.2929x; 1.2929x over previous
"""Expert-parallel MoE (top-2 of 8 experts) for 8 Trainium2 NeuronCores.

Problem (hardcoded): x [4,2048,1024] f32, D=1024, H=4096, E=8, K=2.
Sharding: core c owns token shard c (1024 tokens: routing, dispatch,
final combine) AND expert c (FFN weights W1[c]/b1[c]/W2[c]/b2[c]).

Per-core pipeline:
  1. Load + PE-transpose own x shard; fp32 router matmul -> logits;
     softmax -> gating output; max8 -> top-2 idx/vals; renormalized
     top-2 probs; per-expert prefix-sum (triangular matmuls) gives each
     (token, k) a position pos in its expert's dispatch block.
  2. Dispatch is split into sub-buffers A (pos < CPA) and B (pos >=
     CPA) so the two AllToAlls pipeline with FFN compute. Indirect
     scatters of x rows use bounds-check skip to route each (token, k)
     into exactly one of send1a/send1b.
  3. FFN (fp32r matmuls): blocks 0-2 over recv1a -> send2a, A2A back
     (overlaps blocks 3-4 over recv1b -> send2b), A2A back.
  4. Gather own tokens' two y rows (from recv2a or recv2b at known
     slots, complementary skip-gathers); out = p1*y1 + p2*y2.

Outputs per core: out shard, gating shard, top-idx shard; host concats.
Pad slots in dispatch buffers are never written and never consumed:
garbage there stays column/row-local through both matmuls.
"""

import numpy as np

import concourse.bacc as bacc
import concourse.bass as bass
import concourse.mybir as mybir
from concourse.bass import IndirectOffsetOnAxis
from concourse.tile import TileContext

P = 128
D = 1024
H = 4096
E = 8
NSH = 1024          # tokens per core (shard)
NT = NSH // P       # 8 token tiles per shard
DT = D // P         # 8 contraction tiles over D
HT = H // P         # 32 tiles over H
CPA = 192           # sub-capacity A per (owner, expert) pair
CPB = 128           # sub-capacity B (CPA + CPB must cover max pair count 311)
CP = CPA + CPB
ROWSA = E * CPA     # 1536 rows in A dispatch buffers (3 ffn blocks)
ROWSB = E * CPB     # 1024 rows in B dispatch buffers (2 ffn blocks)
BLK = 512           # ffn block: tokens per block
NBT = BLK // P      # 4 token tiles per block
NBLKA = ROWSA // BLK  # 3
NBLKB = ROWSB // BLK  # 2
DH = 512            # D-half for matmul2 moving N
WG = 4              # h-tiles per W1 wide load
BIG = 1.0e9

F32 = mybir.dt.float32
F32R = mybir.dt.float32r
I32 = mybir.dt.int32
U32 = mybir.dt.uint32
AX = mybir.AxisListType
ALU = mybir.AluOpType
ACT = mybir.ActivationFunctionType


def r(ap):
    return ap.bitcast(F32R)


def build_nc():
    nc = bacc.Bacc(None, target_bir_lowering=False)

    x_sh = nc.dram_tensor("x_shard", [NSH, D], F32, kind="ExternalInput")
    rw = nc.dram_tensor("router_W", [D, E], F32, kind="ExternalInput")
    rb = nc.dram_tensor("router_b", [1, E], F32, kind="ExternalInput")
    w1 = nc.dram_tensor("W1", [D, H], F32, kind="ExternalInput")
    b1 = nc.dram_tensor("b1", [H], F32, kind="ExternalInput")
    w2 = nc.dram_tensor("W2", [H, D], F32, kind="ExternalInput")
    b2 = nc.dram_tensor("b2", [1, D], F32, kind="ExternalInput")

    out_sh = nc.dram_tensor("out_shard", [NSH, D], F32, kind="ExternalOutput")
    gat_sh = nc.dram_tensor("gating_shard", [NSH, E], F32, kind="ExternalOutput")
    tid_sh = nc.dram_tensor("topidx_shard", [NSH, 2], I32, kind="ExternalOutput")

    send1a = nc.dram_tensor("send1a", [ROWSA, D], F32)
    recv1a = nc.dram_tensor("recv1a", [ROWSA, D], F32)
    send1b = nc.dram_tensor("send1b", [ROWSB, D], F32)
    recv1b = nc.dram_tensor("recv1b", [ROWSB, D], F32)
    send2a = nc.dram_tensor("send2a", [ROWSA, D], F32)
    recv2a = nc.dram_tensor("recv2a", [ROWSA, D], F32)
    send2b = nc.dram_tensor("send2b", [ROWSB, D], F32)
    recv2b = nc.dram_tensor("recv2b", [ROWSB, D], F32)

    ident_c = nc.inline_tensor(np.eye(P, dtype=np.float32), name="ident")
    # lt[j, i] = 1 iff j < i : (lt.T @ m)[i] = sum_{j<i} m[j]  (exclusive)
    lt_c = nc.inline_tensor(np.triu(np.ones((P, P), np.float32), 1), name="lt")
    blk = np.zeros((E * NT, E * NT), np.float32)
    for e in range(E):
        s = slice(e * NT, (e + 1) * NT)
        blk[s, s] = np.triu(np.ones((NT, NT), np.float32), 1)
    blk_c = nc.inline_tensor(blk, name="blktri")
    ones_row_c = nc.inline_tensor(np.ones((1, P), np.float32), name="ones_row")
    ones_col_c = nc.inline_tensor(np.ones((P, 1), np.float32), name="ones_col")

    rg = [list(range(8))]

    with TileContext(nc) as tc:
        with (
            tc.tile_pool(name="const", bufs=1) as cpool,
            tc.tile_pool(name="small", bufs=1) as spool,
            tc.tile_pool(name="tps", bufs=2, space="PSUM") as tpsum,
        ):
            # ---- constants into SBUF ----
            id_sb = cpool.tile_from(ident_c[:, :])
            lt_sb = cpool.tile_from(lt_c[:, :])
            blk_sb = cpool.tile_from(blk_c[:, :])
            ones_row = cpool.tile_from(ones_row_c[:, :])
            ones_col = cpool.tile_from(ones_col_c[:, :])
            rb_sb = cpool.tile_from(rb[:, :])
            b2_sb = cpool.tile([1, D], F32R)
            nc.sync.dma_start(b2_sb[:, :], r(b2[:, :]))
            ones_row_r = cpool.tile([1, P], F32R)
            nc.sync.dma_start(ones_row_r[:, :], r(ones_row_c[:, :]))
            rw_sb = cpool.tile([P, DT * E], F32)
            nc.sync.dma_start(rw_sb[:, :], rw.rearrange("(dt p) e -> p dt e", p=P))
            b1_sb = cpool.tile([P, HT], F32)
            nc.sync.dma_start(b1_sb[:, :], b1.rearrange("(ht p) -> p ht", p=P))

            # ---- persistent routing state (alive through phase 5) ----
            p1_sb = spool.tile([P, NT], F32, tag="p1")
            p2_sb = spool.tile([P, NT], F32, tag="p2")
            idx1f = spool.tile([P, NT], F32, tag="idx1f")
            idx2f = spool.tile([P, NT], F32, tag="idx2f")
            mask_sb = spool.tile([P, E * NT], F32, tag="mask")
            sa1_i = spool.tile([P, NT], I32, tag="sa1")  # A slot (or big) k=1
            sb1_i = spool.tile([P, NT], I32, tag="sb1")  # B slot (or big) k=1
            sa2_i = spool.tile([P, NT], I32, tag="sa2")
            sb2_i = spool.tile([P, NT], I32, tag="sb2")

            # =======================================================
            # Phase 1: load + transpose x shard, router, routing state
            # =======================================================
            with (
                tc.tile_pool(name="ph1", bufs=1) as xpool,
                tc.tile_pool(name="ph1s", bufs=2) as wpool,
                tc.tile_pool(name="mmps", bufs=1, space="PSUM") as mpsum,
            ):
                x_tiles = []
                xT = [
                    xpool.tile([P, NSH], F32, tag=f"xT{d}", name=f"xT{d}")
                    for d in range(DT)
                ]
                for t in range(NT):
                    xt = xpool.tile([P, D], F32, tag=f"x{t}")
                    x_tiles.append(xt)
                    nc.sync.dma_start(xt[:, :], x_sh[t * P:(t + 1) * P, :])
                    for d in range(DT):
                        ps = tpsum.tile([P, P], F32, tag="tps", name="tp")
                        nc.tensor.transpose(
                            ps[:, :], xt[:, d * P:(d + 1) * P], id_sb[:, :]
                        )
                        nc.vector.tensor_copy(xT[d][:, t * P:(t + 1) * P], ps[:, :])

                for t in range(NT):
                    tsl = slice(t * P, (t + 1) * P)
                    lps = mpsum.tile([P, E], F32, tag="lps")
                    for d in range(DT):
                        nc.tensor.matmul(
                            lps[:, :],
                            lhsT=xT[d][:, tsl],
                            rhs=rw_sb[:, d * E:(d + 1) * E],
                            start=(d == 0),
                            stop=False,
                        )
                    nc.tensor.matmul(
                        lps[:, :], lhsT=ones_row[:1, :], rhs=rb_sb[:1, :],
                        start=False, stop=True,
                    )
                    logit = wpool.tile([P, E], F32, tag="logit")
                    nc.vector.tensor_copy(logit[:, :], lps[:, :])

                    vals = wpool.tile([P, 8], F32, tag="vals")
                    idxs = wpool.tile([P, 8], U32, tag="idxs")
                    nc.vector.max_with_indices(vals[:, :], idxs[:, :], logit[:, :])

                    # gating_probs = softmax(logits) ; max is vals[:, 0]
                    nm1 = wpool.tile([P, 1], F32, tag="nm1")
                    nc.vector.tensor_scalar_mul(nm1[:, :], vals[:, 0:1], -1.0)
                    exps = wpool.tile([P, E], F32, tag="exps")
                    nc.scalar.activation(
                        exps[:, :], logit[:, :], ACT.Exp, bias=nm1[:, :1], scale=1.0
                    )
                    ssum = wpool.tile([P, 1], F32, tag="ssum")
                    nc.vector.reduce_sum(ssum[:, :], exps[:, :], axis=AX.X)
                    rsum = wpool.tile([P, 1], F32, tag="rsum")
                    nc.vector.reciprocal(rsum[:, :], ssum[:, :])
                    gat = wpool.tile([P, E], F32, tag="gat")
                    nc.vector.tensor_scalar_mul(gat[:, :], exps[:, :], rsum[:, :1])
                    nc.sync.dma_start(gat_sh[tsl, :], gat[:, :])

                    tid = wpool.tile([P, 2], I32, tag="tid")
                    nc.vector.tensor_copy(tid[:, :], idxs[:, 0:2])
                    nc.sync.dma_start(tid_sh[tsl, :], tid[:, :])

                    # renormalized top-2 probs: e2 = exp(v2 - v1)
                    dif = wpool.tile([P, 1], F32, tag="dif")
                    nc.vector.tensor_tensor(
                        dif[:, :], vals[:, 1:2], vals[:, 0:1], op=ALU.subtract
                    )
                    e2 = wpool.tile([P, 1], F32, tag="e2")
                    nc.scalar.activation(e2[:, :], dif[:, :], ACT.Exp)
                    den = wpool.tile([P, 1], F32, tag="den")
                    nc.vector.tensor_scalar_add(den[:, :], e2[:, :], 1.0)
                    rden = wpool.tile([P, 1], F32, tag="rden")
                    nc.vector.reciprocal(rden[:, :], den[:, :])
                    nc.vector.tensor_copy(p1_sb[:, t:t + 1], rden[:, :])
                    nc.vector.tensor_tensor(
                        p2_sb[:, t:t + 1], e2[:, :], rden[:, :], op=ALU.mult
                    )
                    nc.vector.tensor_copy(idx1f[:, t:t + 1], idxs[:, 0:1])
                    nc.vector.tensor_copy(idx2f[:, t:t + 1], idxs[:, 1:2])

                # masks: mask[:, e*NT + t] = (idx1==e) + (idx2==e)
                for e in range(E):
                    esl = slice(e * NT, (e + 1) * NT)
                    eq1 = wpool.tile([P, NT], F32, tag="eq1")
                    eq2 = wpool.tile([P, NT], F32, tag="eq2")
                    nc.vector.tensor_scalar(
                        eq1[:, :], idx1f[:, :], float(e), None, op0=ALU.is_equal
                    )
                    nc.vector.tensor_scalar(
                        eq2[:, :], idx2f[:, :], float(e), None, op0=ALU.is_equal
                    )
                    nc.vector.tensor_tensor(
                        mask_sb[:, esl], eq1[:, :], eq2[:, :], op=ALU.add
                    )

                # prefix sums -> pos (exclusive, token order n = t*128 + p)
                cs_ps = mpsum.tile([E * NT, 1], F32, tag="seq", name="cs")
                nc.tensor.matmul(
                    cs_ps[:, :], lhsT=mask_sb[:, :], rhs=ones_col[:, :1],
                    start=True, stop=True,
                )
                cs_sb = wpool.tile([E * NT, 1], F32, tag="cs_sb")
                nc.vector.tensor_copy(cs_sb[:, :], cs_ps[:, :])
                co_ps = mpsum.tile([E * NT, 1], F32, tag="seq", name="co")
                nc.tensor.matmul(
                    co_ps[:, :], lhsT=blk_sb[:, :], rhs=cs_sb[:, :1],
                    start=True, stop=True,
                )
                co_sb = wpool.tile([E * NT, 1], F32, tag="co_sb")
                nc.vector.tensor_copy(co_sb[:, :], co_ps[:, :])
                cot_ps = mpsum.tile([1, E * NT], F32, tag="seq", name="cot")
                nc.tensor.transpose(
                    cot_ps[:1, :], co_sb[:, :1], id_sb[:E * NT, :E * NT]
                )
                cot_sb = wpool.tile([1, E * NT], F32, tag="cot_sb")
                nc.vector.tensor_copy(cot_sb[:1, :], cot_ps[:1, :])

                pos_ps = mpsum.tile([P, E * NT], F32, tag="pos")
                nc.tensor.matmul(
                    pos_ps[:, :], lhsT=lt_sb[:, :], rhs=mask_sb[:, :],
                    start=True, stop=False,
                )
                nc.tensor.matmul(
                    pos_ps[:, :], lhsT=ones_row[:1, :], rhs=cot_sb[:1, :],
                    start=False, stop=True,
                )
                pos_sb = wpool.tile([P, E * NT], F32, tag="pos_sb")
                nc.vector.tensor_copy(pos_sb[:, :], pos_ps[:, :])

                # slots: pos_k = pos[:, idx_k block]; clamped to CP-1.
                # A slot = idx_k*CPA + pos_k       if pos_k <  CPA else BIG
                # B slot = idx_k*CPB + pos_k - CPA if pos_k >= CPA else BIG
                for idxf, sa_i, sb_i in (
                    (idx1f, sa1_i, sb1_i),
                    (idx2f, sa2_i, sb2_i),
                ):
                    posk = wpool.tile([P, NT], F32, tag="posk")
                    nc.vector.memset(posk[:, :], 0.0)
                    for e in range(E):
                        esl = slice(e * NT, (e + 1) * NT)
                        eq = wpool.tile([P, NT], F32, tag="eq")
                        nc.vector.tensor_scalar(
                            eq[:, :], idxf[:, :], float(e), None, op0=ALU.is_equal
                        )
                        pe = wpool.tile([P, NT], F32, tag="pe")
                        nc.vector.tensor_tensor(
                            pe[:, :], eq[:, :], pos_sb[:, esl], op=ALU.mult
                        )
                        nc.vector.tensor_tensor(
                            posk[:, :], posk[:, :], pe[:, :], op=ALU.add
                        )
                    nc.vector.tensor_scalar_min(posk[:, :], posk[:, :], float(CP - 1))

                    cmpa = wpool.tile([P, NT], U32, tag="cmpa")
                    nc.vector.tensor_scalar(
                        cmpa[:, :], posk[:, :], float(CPA), None, op0=ALU.is_lt
                    )
                    basea = wpool.tile([P, NT], F32, tag="basea")
                    nc.vector.tensor_scalar(
                        basea[:, :], idxf[:, :], float(CPA), None, op0=ALU.mult
                    )
                    nc.vector.tensor_tensor(
                        basea[:, :], basea[:, :], posk[:, :], op=ALU.add
                    )
                    saf = wpool.tile([P, NT], F32, tag="saf")
                    nc.vector.memset(saf[:, :], BIG)
                    nc.vector.copy_predicated(saf[:, :], cmpa[:, :], basea[:, :])
                    nc.vector.tensor_copy(sa_i[:, :], saf[:, :])

                    cmpb = wpool.tile([P, NT], U32, tag="cmpb")
                    nc.vector.tensor_scalar(
                        cmpb[:, :], posk[:, :], float(CPA), None, op0=ALU.is_ge
                    )
                    baseb = wpool.tile([P, NT], F32, tag="baseb")
                    nc.vector.tensor_scalar(
                        baseb[:, :], idxf[:, :], float(CPB), float(-CPA),
                        op0=ALU.mult, op1=ALU.add,
                    )
                    nc.vector.tensor_tensor(
                        baseb[:, :], baseb[:, :], posk[:, :], op=ALU.add
                    )
                    sbf = wpool.tile([P, NT], F32, tag="sbf")
                    nc.vector.memset(sbf[:, :], BIG)
                    nc.vector.copy_predicated(sbf[:, :], cmpb[:, :], baseb[:, :])
                    nc.vector.tensor_copy(sb_i[:, :], sbf[:, :])

                # dispatch: scatter x rows (skip out-of-range via bounds)
                for t in range(NT):
                    for s_i, buf, nrows in (
                        (sa1_i, send1a, ROWSA),
                        (sa2_i, send1a, ROWSA),
                        (sb1_i, send1b, ROWSB),
                        (sb2_i, send1b, ROWSB),
                    ):
                        nc.gpsimd.indirect_dma_start(
                            out=buf[:, :],
                            out_offset=IndirectOffsetOnAxis(
                                ap=s_i[:, t:t + 1], axis=0
                            ),
                            in_=x_tiles[t][:, :],
                            in_offset=None,
                            bounds_check=nrows - 1,
                            oob_is_err=False,
                        )

            nc.gpsimd.collective_compute(
                "AllToAll", ALU.bypass, ins=[send1a[:, :]], outs=[recv1a[:, :]],
                replica_groups=rg,
            )
            nc.gpsimd.collective_compute(
                "AllToAll", ALU.bypass, ins=[send1b[:, :]], outs=[recv1b[:, :]],
                replica_groups=rg,
            )

            # =======================================================
            # Phase 3: FFN over recv1 rows (fp32r)
            # =======================================================
            with (
                tc.tile_pool(name="xg", bufs=3) as xgpool,
                tc.tile_pool(name="xgt", bufs=2) as xgtpool,
                tc.tile_pool(name="ht", bufs=1) as htpool,
                tc.tile_pool(name="w1p", bufs=3) as w1pool,
                tc.tile_pool(name="w2p", bufs=6) as w2pool,
                tc.tile_pool(name="yp", bufs=4) as ypool,
                tc.tile_pool(name="ps1", bufs=2, space="PSUM") as ps1pool,
                tc.tile_pool(name="ps2", bufs=1, space="PSUM") as ps2pool,
            ):
                def ffn_block(rbuf, sbuf, base):
                    xgT = [
                        xgtpool.tile([P, BLK], F32R, tag=f"xgT{d}", name=f"xgT{d}")
                        for d in range(DT)
                    ]
                    for i in range(NBT):
                        xg = xgpool.tile([P, D], F32, tag="xg")
                        nc.sync.dma_start(
                            xg[:, :], rbuf[base + i * P:base + (i + 1) * P, :]
                        )
                        for d in range(DT):
                            ps = tpsum.tile([P, P], F32, tag="tps", name="tp")
                            nc.tensor.transpose(
                                ps[:, :], xg[:, d * P:(d + 1) * P], id_sb[:, :]
                            )
                            nc.vector.tensor_copy(
                                xgT[d][:, i * P:(i + 1) * P], ps[:, :]
                            )

                    hts = [
                        htpool.tile([P, BLK], F32R, tag=f"hT{h}", name=f"hT{h}")
                        for h in range(HT)
                    ]
                    for hg in range(HT // WG):
                        w1t = []
                        for d in range(DT):
                            wt = w1pool.tile(
                                [P, WG * P], F32R, tag=f"w1_{d}", name=f"w1_{d}"
                            )
                            nc.sync.dma_start(
                                wt[:, :],
                                r(w1[
                                    d * P:(d + 1) * P,
                                    hg * WG * P:(hg + 1) * WG * P,
                                ]),
                            )
                            w1t.append(wt)
                        for hh in range(WG):
                            h = hg * WG + hh
                            ps = ps1pool.tile([P, BLK], F32, tag="mm1")
                            for d in range(DT):
                                nc.tensor.matmul(
                                    ps[:, :],
                                    lhsT=w1t[d][:, hh * P:(hh + 1) * P],
                                    rhs=xgT[d][:, :],
                                    start=(d == 0),
                                    stop=(d == DT - 1),
                                )
                            nc.scalar.activation(
                                hts[h][:, :], ps[:, :], ACT.Relu,
                                bias=b1_sb[:, h:h + 1], scale=1.0,
                            )

                    for dh in range(D // DH):
                        dsl = slice(dh * DH, (dh + 1) * DH)
                        yps = [
                            ps2pool.tile([P, DH], F32, tag=f"ys{i}", name=f"ys{i}")
                            for i in range(NBT)
                        ]
                        for h in range(HT):
                            w2t = w2pool.tile([P, DH], F32R, tag="w2")
                            nc.sync.dma_start(
                                w2t[:, :], r(w2[h * P:(h + 1) * P, dsl])
                            )
                            for i in range(NBT):
                                nc.tensor.matmul(
                                    yps[i][:, :],
                                    lhsT=hts[h][:, i * P:(i + 1) * P],
                                    rhs=w2t[:, :],
                                    start=(h == 0),
                                    stop=False,
                                )
                        for i in range(NBT):
                            nc.tensor.matmul(
                                yps[i][:, :], lhsT=ones_row_r[:1, :],
                                rhs=b2_sb[:1, dsl], start=False, stop=True,
                            )
                            yt = ypool.tile([P, DH], F32, tag="y")
                            nc.vector.tensor_copy(yt[:, :], yps[i][:, :])
                            nc.sync.dma_start(
                                sbuf[base + i * P:base + (i + 1) * P, dsl],
                                yt[:, :],
                            )

                for b in range(NBLKA):
                    ffn_block(recv1a, send2a, b * BLK)

                nc.gpsimd.collective_compute(
                    "AllToAll", ALU.bypass, ins=[send2a[:, :]],
                    outs=[recv2a[:, :]], replica_groups=rg,
                )

                for b in range(NBLKB):
                    ffn_block(recv1b, send2b, b * BLK)

            nc.gpsimd.collective_compute(
                "AllToAll", ALU.bypass, ins=[send2b[:, :]], outs=[recv2b[:, :]],
                replica_groups=rg,
            )

            # =======================================================
            # Phase 5: combine own tokens
            # =======================================================
            with tc.tile_pool(name="comb", bufs=3) as opool:
                for t in range(NT):
                    tsl = slice(t * P, (t + 1) * P)
                    y1 = opool.tile([P, D], F32, tag="y1")
                    y2 = opool.tile([P, D], F32, tag="y2")
                    for yt_, sa_i, sb_i in ((y1, sa1_i, sb1_i), (y2, sa2_i, sb2_i)):
                        nc.gpsimd.indirect_dma_start(
                            out=yt_[:, :], out_offset=None, in_=recv2a[:, :],
                            in_offset=IndirectOffsetOnAxis(
                                ap=sa_i[:, t:t + 1], axis=0
                            ),
                            bounds_check=ROWSA - 1,
                            oob_is_err=False,
                        )
                        nc.gpsimd.indirect_dma_start(
                            out=yt_[:, :], out_offset=None, in_=recv2b[:, :],
                            in_offset=IndirectOffsetOnAxis(
                                ap=sb_i[:, t:t + 1], axis=0
                            ),
                            bounds_check=ROWSB - 1,
                            oob_is_err=False,
                        )
                    nc.vector.tensor_scalar_mul(y1[:, :], y1[:, :], p1_sb[:, t:t + 1])
                    nc.vector.tensor_scalar_mul(y2[:, :], y2[:, :], p2_sb[:, t:t + 1])
                    nc.vector.tensor_tensor(
                        y1[:, :], y1[:, :], y2[:, :], op=ALU.add
                    )
                    nc.sync.dma_start(out_sh[tsl, :], y1[:, :])

    nc.compile()
    return nc


_NC_CACHE = None


def _get_nc():
    global _NC_CACHE
    if _NC_CACHE is None:
        _NC_CACHE = build_nc()
    return _NC_CACHE


def make_in_maps(x, router_W, router_b, W1, b1, W2, b2):
    x = np.ascontiguousarray(np.asarray(x, dtype=np.float32)).reshape(8 * NSH, D)
    router_W = np.ascontiguousarray(np.asarray(router_W, dtype=np.float32))
    router_b = np.ascontiguousarray(
        np.asarray(router_b, dtype=np.float32).reshape(1, E)
    )
    W1 = np.ascontiguousarray(np.asarray(W1, dtype=np.float32))
    b1 = np.ascontiguousarray(np.asarray(b1, dtype=np.float32))
    W2 = np.ascontiguousarray(np.asarray(W2, dtype=np.float32))
    b2 = np.ascontiguousarray(np.asarray(b2, dtype=np.float32))
    in_maps = []
    for c in range(8):
        in_maps.append({
            "x_shard": x[c * NSH:(c + 1) * NSH],
            "router_W": router_W,
            "router_b": router_b,
            "W1": W1[c],
            "b1": b1[c],
            "W2": W2[c],
            "b2": b2[c].reshape(1, D),
        })
    return in_maps


def assemble(results):
    out = np.concatenate([res["out_shard"] for res in results], axis=0)
    gat = np.concatenate([res["gating_shard"] for res in results], axis=0)
    tid = np.concatenate([res["topidx_shard"] for res in results], axis=0)
    return (
        out.reshape(4, 2048, D).astype(np.float32),
        gat.reshape(4, 2048, E).astype(np.float32),
        tid.reshape(4, 2048, 2).astype(np.int32),
    )


def kernel(x, router_W, router_b, W1, b1, W2, b2):
    from concourse.bass_utils import run_bass_kernel_spmd

    nc = _get_nc()
    in_maps = make_in_maps(x, router_W, router_b, W1, b1, W2, b2)
    res = run_bass_kernel_spmd(nc, in_maps, list(range(8)))
    return assemble(res.results)
